# revision 31
# baseline (speedup 1.0000x reference)
"""Multi-head attention layer (B=4, S=2048, HID=1024, 16 heads) on 8 TRN2 NeuronCores.

Sharding (hardcoded): core c -> (batch b = c//2, head-group g = c%2).
Each core computes its 8 heads' full attention for its batch.

v2 architecture: the kernel is ScalarE-bound (256 exp activations of
[128,1024], ~1.1us each, ~288us total).  Everything else hides under
the exp stream:

  - Attention runs as one flat 256-step pipeline (quarter-major,
    pair-major, key-chunk inner) starting as soon as pair 0's Q/K
    projections land (~25us), instead of after ALL projections (~115us
    in v1).  All remaining work (pairs 1-3 K proj, Q proj split into
    s-quarter units by per-quarter deadline, V proj, output-projection
    strips, mask/weight DMAs) is injected between attention steps as
    4-matmul "filler" parts, ordered by data deadline and sized so a
    part never delays the next E by more than the exp runway.  E for
    step i+3 is emitted before step i's fillers; every consumer is
    emitted after its producer (the Tile framework derives
    dependencies from program order, not runtime order).
  - ScalarE does ONLY exp (plus pair-0 projection evacuations and the
    quarter-0 mask-piece DMA issues in the prologue while it is idle).
    bq/bk/bv are identically zero in this problem's setup_inputs
    (asserted host-side).
  - DVE carries the mask multiplies (2x mode), projection/V/strip
    evacuations, reciprocals, and normalize multiplies -- ~250us,
    under the exp stream.  GPSIMD runs ONLY partition_broadcast:
    mixing op families on Pool triggers ~6-9us microcode library swaps
    (measured), so everything else was moved off it.
  - Normalize per pair is split: part A (oacc -> otmp evacuation,
    denominator row via SBUF->SBUF DMA, reciprocal, partition
    broadcast) at the pair's last step; part B (normalize multiplies +
    rows 64-127 via DMA) three steps later so no engine queue
    head-blocks on the chain, and the next pair's PSUM accumulators
    recycle without stalling exp.
  - Tail: the last quarter's output-projection strips accumulate pairs
    0-2 during the final steps, with only pair 3's contribution +
    evacuation (split across DVE/ACT and both DMA queues) after the
    last exp; dummy ps2 matmuls keep the PE p-state up across the
    drain.  PSUM: 3x [128,1024] rotating (E/proj/V/strips) + 2 banks
    for the PV accumulators ([65,512]; 65th V column of ones yields
    the softmax denominators).

Numerics (exact vs the reference up to float rounding): softmax
without max-subtraction (|scores| <= ~8, exp cannot overflow);
exp * {0,1}-mask == the reference's -1e9 masking; bo added on host;
Wo and normalized probabilities in bf16 (measured 6.7e-3 relative
absmax vs the fp32 reference, gate is 2e-2).
"""

import sys

for _p in ("/opt/trn_rl_repo", "/root/.axon_site/_ro/trn_rl_repo"):
    if _p not in sys.path:
        sys.path.insert(0, _p)

import numpy as np
import ml_dtypes

import concourse.bass as bass
import concourse.tile as tile
from concourse import bacc, mybir
from concourse.bass_utils import run_bass_kernel_spmd

F32 = mybir.dt.float32
BF16 = mybir.dt.bfloat16
NPBF16 = ml_dtypes.bfloat16

B, S, HID = 4, 2048, 1024
HEADS, DH = 16, 64
NCORES = 8
D = 512
HLOC = 8
NPAIR = 4
P = 128
KC = S // P      # 16 key chunks
NKP = HID // P   # 8 contraction chunks
SCALE = 1.0 / 8.0
EXP = mybir.ActivationFunctionType.Exp

PE_BUFS = 5       # pe_t (exp output) elasticity
PM_BUFS = 5       # pm (masked probs) elasticity / PV lag tolerance
AT_BUFS = 6
MASK_BUFS = 4     # [P, 4, 512] quarter-piece mask tiles
POOL_MASK_KCS = ()  # Pool mask-mult offload hurt on HW (PV stalls)
PV_DEFER = 2

_CACHED = None


def _build_program():
    nc = bacc.Bacc("TRN2", target_bir_lowering=False, debug=False,
                   num_devices=NCORES)

    xq = nc.dram_tensor("xq", [HID, S], BF16, kind="ExternalInput").ap()
    xk = nc.dram_tensor("xk", [HID, S], BF16, kind="ExternalInput").ap()
    xv = nc.dram_tensor("xv", [HID, S], BF16, kind="ExternalInput").ap()
    mk = nc.dram_tensor("maskT", [S, S], BF16, kind="ExternalInput").ap()
    wq = nc.dram_tensor("wq", [HID, D], BF16, kind="ExternalInput").ap()
    wk = nc.dram_tensor("wk", [HID, D], BF16, kind="ExternalInput").ap()
    wv = nc.dram_tensor("wv", [HID, D], BF16, kind="ExternalInput").ap()
    wo = nc.dram_tensor("wo", [D, HID], BF16, kind="ExternalInput").ap()
    out = nc.dram_tensor("out", [S, HID], F32, kind="ExternalOutput").ap()

    with tile.TileContext(nc) as tc:
        with tc.tile_pool(name="sb", bufs=1) as sb, \
             tc.tile_pool(name="ps", bufs=1, space="PSUM") as ps:

            # ---------------- persistent SBUF ----------------
            qt = [sb.tile([P, S], BF16, tag="qt", bufs=NPAIR, name=f"qt{p}")
                  for p in range(NPAIR)]
            kt = [sb.tile([P, S], BF16, tag="kt", bufs=NPAIR, name=f"kt{p}")
                  for p in range(NPAIR)]
            v_sb = sb.tile([P, KC, HLOC, DH + 1], BF16, tag="v", name="v_sb")
            nc.vector.memset(v_sb[:, :, :, DH:DH + 1], 1.0)
            wo_sb = sb.tile([P, 4, HID], BF16, tag="wo", name="wo_sb")

            # ---------------- prologue DMAs ----------------
            # Split across BOTH hardware DGE queues (SP + the idle
            # Activation queue) so the lead-in halves.  Each FIFO is
            # ordered by data deadline; pool allocations that could
            # block a FIFO sit at its tail.
            w_t = {}

            def load_w(nm, wd, eng):
                t = sb.tile([P, NKP, D], BF16, tag="w", bufs=3, name=nm)
                eng.dma_start(t[:], wd.rearrange("(c p) d -> p c d", p=P))
                w_t[nm] = t

            x_t = {}

            def load_xhalf(key, xd, sh, eng, split=False):
                t = sb.tile([P, NKP, 1024], BF16, tag="x", bufs=3,
                            name=f"{key}h{sh}")
                view = xd.rearrange("(c p) s -> p c s", p=P)
                if split:
                    # two k-half transfers so the projection's first
                    # parts can start while the second half streams
                    eng.dma_start(t[:, 0:4, :],
                                  view[:, 0:4, sh * 1024:(sh + 1) * 1024])
                    eng.dma_start(t[:, 4:8, :],
                                  view[:, 4:8, sh * 1024:(sh + 1) * 1024])
                else:
                    eng.dma_start(t[:],
                                  view[:, :, sh * 1024:(sh + 1) * 1024])
                x_t[(key, sh)] = t

            masks = {}

            def load_mask(qh, qq, piece, eng):
                mt = sb.tile([P, 4, 512], BF16, tag="mask", bufs=MASK_BUFS,
                             name="mask_sb")
                eng.dma_start(
                    mt[:],
                    mk.rearrange("(kc p) (u q) -> u p kc q", p=P, q=512)
                    [qh * 2 + qq][:, piece * 4:(piece + 1) * 4, :])
                masks[(qh, qq, piece)] = mt

            xv_t = {}

            def load_xv(g, eng):
                t = sb.tile([P, NKP, 512], BF16, tag="xv", bufs=2,
                            name=f"xv{g}")
                eng.dma_start(
                    t[:], xv.rearrange("(c p) s -> p c s", p=P)
                    [:, :, g * 512:(g + 1) * 512])
                xv_t[g] = t

            # SP queue, deadline-ordered.  ACT-queue DMA issues cost
            # ~2.3us of ScalarE each, so only the quarter-0 mask pieces
            # ride that queue (issued before the first exp, when the
            # engine is otherwise idle anyway).
            load_w("wk", wk, nc.sync)
            load_w("wq", wq, nc.sync)
            load_xhalf("xk", xk, 0, nc.sync)
            load_xhalf("xq", xq, 0, nc.sync)
            load_w("wv", wv, nc.sync)
            load_xv(0, nc.sync)
            load_xhalf("xk", xk, 1, nc.sync)
            load_xv(1, nc.sync)
            for piece in range(4):
                load_mask(0, 0, piece, nc.scalar)

            # ---------------- work units ----------------
            proj_state = {}

            def proj_part(wkey, xkey, dst, m, sh, part, evac="dve"):
                """Quarter of a (pair m, s-half sh) projection: 4
                matmuls (n2 = part//2, k-half = part%2); part 3
                evacuates the [128, 1024] tile to dst[m] (bf16)."""
                if part == 0:
                    proj_state[(wkey, m, sh)] = ps.tile(
                        [P, 1024], F32, tag="ps4", bufs=3, name="prps")
                acc = proj_state[(wkey, m, sh)]
                n2, kh = part // 2, part % 2
                for k in range(kh * 4, kh * 4 + 4):
                    nc.tensor.matmul(
                        acc[:, n2 * 512:(n2 + 1) * 512],
                        lhsT=w_t[wkey][:, k, m * P:(m + 1) * P],
                        rhs=x_t[(xkey, sh)][:, k, n2 * 512:(n2 + 1) * 512],
                        start=(k == 0), stop=(k == NKP - 1))
                if part == 3:
                    dstap = dst[m][:, sh * 1024:(sh + 1) * 1024]
                    if evac == "act":
                        nc.scalar.copy(dstap, acc[:])
                    else:
                        nc.vector.tensor_copy(dstap, acc[:])
                    del proj_state[(wkey, m, sh)]

            def proj_sq(wkey, xkey, dst, m, sh, sq, part, evac="dve"):
                """s-quarter Q projection (4 matmuls per part; 2 parts):
                512 output columns, own psum tile, for split deadlines."""
                key = (wkey, m, sh, sq)
                if part == 0:
                    proj_state[key] = ps.tile([P, 1024], F32, tag="ps4",
                                              bufs=3, name="prps")
                acc = proj_state[key]
                for k in range(part * 4, part * 4 + 4):
                    nc.tensor.matmul(
                        acc[:, 0:512],
                        lhsT=w_t[wkey][:, k, m * P:(m + 1) * P],
                        rhs=x_t[(xkey, sh)][:, k,
                                            sq * 512:(sq + 1) * 512],
                        start=(k == 0), stop=(k == NKP - 1))
                if part == 1:
                    c0 = sh * 1024 + sq * 512
                    dstap = dst[m][:, c0:c0 + 512]
                    if evac == "act":
                        nc.scalar.copy(dstap, acc[:, 0:512])
                    else:
                        nc.vector.tensor_copy(dstap, acc[:, 0:512])
                    del proj_state[key]

            v_ps = {}

            def v_chunk_part(m, kh):
                """Half of V-projection s-chunk m (4 matmuls); kh==1
                evacuates the chunk (PV of step kc=m reads it)."""
                slot, half = m // 2, m % 2
                if half == 0 and kh == 0:
                    v_ps[slot] = ps.tile([P, 1024], F32, tag="ps4", bufs=3,
                                         name="vps")
                accv = v_ps[slot]
                g, part = m // 4, m % 4
                for k in range(kh * 4, kh * 4 + 4):
                    nc.tensor.matmul(
                        accv[:, half * 512:(half + 1) * 512],
                        lhsT=xv_t[g][:, k, part * P:(part + 1) * P],
                        rhs=w_t["wv"][:, k, :],
                        start=(k == 0), stop=(k == NKP - 1))
                if kh == 1:
                    nc.vector.tensor_copy(
                        v_sb[:, m, :, 0:DH],
                        accv[:, half * 512:(half + 1) * 512]
                        .rearrange("p (h d) -> p h d", h=HLOC))
                    if half == 1:
                        del v_ps[slot]

            strip_state = {}

            def outproj_part(qh, qq, at4, m, kh):
                q0 = qh * 1024 + qq * 512
                if kh == 0:
                    strip_state[(qh, qq, m)] = ps.tile(
                        [P, HID], F32, tag="ps4", bufs=3, name="ops")
                ops = strip_state[(qh, qq, m)]
                for k in range(kh * 2, kh * 2 + 2):
                    for n2 in range(2):
                        nc.tensor.matmul(
                            ops[:, n2 * 512:(n2 + 1) * 512],
                            lhsT=at4[k][:, m * P:(m + 1) * P],
                            rhs=wo_sb[:, k, n2 * 512:(n2 + 1) * 512],
                            start=(k == 0), stop=(k == 3))
                if kh == 1:
                    ost = sb.tile([P, HID], F32, tag="ost", bufs=2,
                                  name="ost")
                    nc.vector.tensor_copy(ost[:], ops[:])
                    nc.sync.dma_start(out[q0 + m * P: q0 + (m + 1) * P, :],
                                      ost[:])
                    del strip_state[(qh, qq, m)]

            def outproj_partial3(m, at3):
                """Last-quarter strip m: pairs 0-2 accumulation (6
                matmuls), emitted during the final steps once E
                allocations have ceased (ps4 rotation)."""
                ops = ps.tile([P, HID], F32, tag="ps4", bufs=3, name="ops")
                strip_state[("last", m)] = ops
                for k in range(3):
                    for n2 in range(2):
                        nc.tensor.matmul(
                            ops[:, n2 * 512:(n2 + 1) * 512],
                            lhsT=at3[k][:, m * P:(m + 1) * P],
                            rhs=wo_sb[:, k, n2 * 512:(n2 + 1) * 512],
                            start=(k == 0), stop=False,
                            skip_group_check=True)

            def outproj_final(m, at_last):
                q0 = 1024 + 512
                ops = strip_state.pop(("last", m))
                for n2 in range(2):
                    nc.tensor.matmul(
                        ops[:, n2 * 512:(n2 + 1) * 512],
                        lhsT=at_last[:, m * P:(m + 1) * P],
                        rhs=wo_sb[:, 3, n2 * 512:(n2 + 1) * 512],
                        start=False, stop=True, skip_group_check=True)
                ost = sb.tile([P, HID], F32, tag="ost", bufs=2, name="ost")
                nc.vector.tensor_copy(ost[:], ops[:])
                nc.sync.dma_start(out[q0 + m * P: q0 + (m + 1) * P, :],
                                  ost[:])

            # ---------------- normalize (split A/B) ----------------
            def normalize_a(oacc):
                otmp = [sb.tile([DH + 1, 512], F32, tag="otmp", bufs=2,
                                name="otmp") for _ in range(2)]
                for hh in range(2):
                    nc.vector.tensor_copy(otmp[hh][:], oacc[hh][:])
                d0 = sb.tile([1, 1024], F32, tag="d0", bufs=1, name="d0")
                for hh in range(2):
                    nc.sync.dma_start(d0[0:1, hh * 512:(hh + 1) * 512],
                                      otmp[hh][DH:DH + 1, :])
                nc.vector.reciprocal_approx_fast(d0[:], d0[:])
                rb = sb.tile([DH, 1024], F32, tag="rb", bufs=1, name="rb")
                nc.gpsimd.partition_broadcast(rb[:], d0[:], channels=DH)
                return otmp, rb

            def normalize_b(otmp, rb, on_pool=False):
                eng = nc.vector
                at = sb.tile([P, 512], BF16, tag="at", bufs=AT_BUFS,
                             name="at")
                eng.tensor_mul(at[0:DH, :], otmp[0][0:DH, :],
                               rb[:, 0:512])
                tb = sb.tile([DH, 512], BF16, tag="tmpb", bufs=1, name="tb")
                eng.tensor_mul(tb[:], otmp[1][0:DH, :],
                               rb[:, 512:1024])
                nc.sync.dma_start(at[DH:P, :], tb[:])
                return at

            # ---------------- step list & E ----------------
            quarters = [(0, 0), (0, 1), (1, 0), (1, 1)]
            steps = [(qh, qq, pr, kc)
                     for (qh, qq) in quarters
                     for pr in range(NPAIR)
                     for kc in range(KC)]
            NSTEP = len(steps)
            LOOKAHEAD = 3
            eps = {}

            def emit_e(qh, qq, pr, kc):
                q0 = qh * 1024 + qq * 512
                ep = ps.tile([P, 1024], F32, tag="ps4", bufs=3, name="ep")
                for hh in range(2):
                    rows = slice(hh * DH, (hh + 1) * DH)
                    nc.tensor.matmul(
                        ep[:, hh * 512:(hh + 1) * 512],
                        lhsT=kt[pr][rows, kc * P:(kc + 1) * P],
                        rhs=qt[pr][rows, q0:q0 + 512],
                        start=True, stop=True)
                eps[(qh, qq, pr, kc)] = ep

            # ---------------- filler schedule ----------------
            from collections import defaultdict
            fill = defaultdict(list)

            def PU(idx, wkey, xkey, dst, m, sh, evac="dve"):
                """Projection unit as 4 single-step parts at idx..idx+3."""
                for part in range(4):
                    fill[idx + part].append(
                        (lambda p: lambda: proj_part(wkey, xkey, dst, m,
                                                     sh, p, evac))(part))

            def QU(idx, m, sh, sq):
                """Q-proj s-quarter unit: 2 parts at idx, idx+1."""
                for part in range(2):
                    fill[idx + part].append(
                        (lambda p: lambda: proj_sq("wq", "xq", qt, m, sh,
                                                   sq, p))(part))

            # V chunk m: parts at steps m-1, m (PV of step kc=m reads the
            # evac; program order defines the dependency).
            fill[0].append(lambda: v_chunk_part(0, 0))
            fill[0].append(lambda: v_chunk_part(0, 1))
            for m in range(1, KC):
                fill[m - 1].append((lambda mm: lambda: v_chunk_part(mm, 0))(m))
                fill[m].append((lambda mm: lambda: v_chunk_part(mm, 1))(m))
            fill[4].append(lambda: load_xv(2, nc.sync))
            fill[10].append(lambda: load_xv(3, nc.sync))
            # pair-0 s-half-1 K proj (E kc8 of pair 0 is emitted at step 5)
            # keys 512:1024 of pair 0 (E kc4 emitted at step 1)
            fill[0].insert(0, lambda: proj_sq("wk", "xk", kt, 0, 0, 1, 0,
                                              "act"))
            fill[0].insert(1, lambda: proj_sq("wk", "xk", kt, 0, 0, 1, 1,
                                              "act"))
            PU(1, "wk", "xk", kt, 0, 1)
            # pairs 1-3 (E of pair p emitted from step 16p-3; kc8 at 16p+5)
            PU(5, "wk", "xk", kt, 1, 0)        # evac @8 < 13
            QU(11, 1, 0, 0)                    # qt1 q-cols 0:512 by 13
            PU(16, "wk", "xk", kt, 1, 1)       # evac @19 < 21
            PU(20, "wk", "xk", kt, 2, 0)       # evac @23 < 29
            QU(26, 2, 0, 0)                    # by 29
            PU(30, "wk", "xk", kt, 2, 1)       # evac @33 < 37
            PU(36, "wk", "xk", kt, 3, 0)       # evac @39 < 45
            QU(42, 3, 0, 0)                    # by 45
            PU(47, "wk", "xk", kt, 3, 1)       # evac @50 < 53
            # deferred q-cols 512:1024 (quarter (0,1), deadlines 61+16p)
            QU(22, 0, 0, 1)
            QU(33, 1, 0, 1)
            QU(56, 2, 0, 1)
            QU(70, 3, 0, 1)
            # bulk mid-run loads placed in DMA-quiet windows between the
            # per-pair normalize chains
            fill[52].append(lambda: load_xhalf("xq", xq, 1, nc.sync))
            fill[48].append(lambda: nc.sync.dma_start(
                wo_sb[:], wo.rearrange("(c p) n -> p c n", p=P)))
            # deferred Q proj s-half 1 in s-quarters (quarter (1,0)
            # needs q-cols 1024:1536 from step 125+16p; (1,1) cols
            # 1536:2048 from 189+16p)
            for i in range(NPAIR):
                QU(94 + 6 * i, i, 1, 0)
                QU(150 + 6 * i, i, 1, 1)
            # mask quarter-pieces for quarters 1-3 (slot of piece j of
            # the prior quarter frees at step 64(Q-1)+51+4j)
            for Q in range(1, 4):
                qh_, qq_ = quarters[Q]
                for j in range(4):
                    fill[64 * Q - 12 + 4 * j].append(
                        (lambda a, b, c: lambda: load_mask(a, b, c, nc.sync))
                        (qh_, qq_, j))

            # ---------------- prologue PE work ----------------
            for part in range(2):
                proj_sq("wk", "xk", kt, 0, 0, 0, part, "act")
            for part in range(2):
                proj_sq("wq", "xq", qt, 0, 0, 0, part, "act")

            # ---------------- main loop ----------------
            oaccs = {}
            ats = {}
            pending = defaultdict(list)

            for j in range(LOOKAHEAD):
                emit_e(*steps[j])

            def emit_pv(pr, kc, pm_t):
                for hh in range(2):
                    nc.tensor.matmul(
                        oaccs[pr][hh][:],
                        lhsT=v_sb[:, kc, 2 * pr + hh, :],
                        rhs=pm_t[:, hh, :],
                        start=(kc == 0), stop=(kc == KC - 1),
                        skip_group_check=True)

            for i, (qh, qq, pr, kc) in enumerate(steps):
                # E first: filler psum-allocation stalls then only delay
                # E(i+4..), absorbed by the lookahead.  All qt/kt/mask
                # producers are scheduled >= 1 step before the first E
                # emission that reads them.
                if i + LOOKAHEAD < NSTEP:
                    emit_e(*steps[i + LOOKAHEAD])
                for fn in fill.pop(i, ()):
                    fn()
                for fn in pending.pop(i, ()):
                    fn()

                if kc == 0:
                    oaccs[pr] = [ps.tile([DH + 1, 512], F32, tag="ps2",
                                         bufs=2, name="oacc")
                                 for _ in range(2)]

                ep = eps.pop((qh, qq, pr, kc))
                pe_t = sb.tile([P, 1024], BF16, tag="p", bufs=PE_BUFS,
                               name="pexp")
                nc.scalar.activation(pe_t[:], ep[:], EXP, scale=SCALE)
                pm_t = sb.tile([P, 2, 512], BF16, tag="pm", bufs=PM_BUFS,
                               name="pmask")
                mslice = masks[(qh, qq, kc // 4)][:, kc % 4, :]
                eng = (nc.gpsimd if kc in POOL_MASK_KCS else nc.vector)
                eng.tensor_mul(
                    pm_t[:],
                    pe_t[:].rearrange("p (h q) -> p h q", h=2),
                    mslice.unsqueeze(1).to_broadcast([P, 2, 512]))
                if kc in POOL_MASK_KCS:
                    pending[i + PV_DEFER].append(
                        (lambda c, d, t: lambda: emit_pv(c, d, t))
                        (pr, kc, pm_t))
                else:
                    emit_pv(pr, kc, pm_t)

                if kc == KC - 1:
                    # defer A one step so its DVE burst (2 copies +
                    # reciprocal) doesn't collide with this boundary's
                    # mask multiplies; B follows at +4.
                    oacc_done = oaccs.pop(pr)

                    def mk_b(o, r, q_h, q_q, p_r, base):
                        last_q = (q_h, q_q) == quarters[-1]

                        def go():
                            at = normalize_b(o, r,
                                             on_pool=not (last_q and
                                                          p_r == 3))
                            ats.setdefault((q_h, q_q), []).append(at)
                            if last_q and p_r == 2:
                                # strips 0-2 partial (pairs 0-2) in the
                                # last 3 steps; strip 3's partial waits
                                # for a ps4 slot freed by final(0).
                                at3 = list(ats[(q_h, q_q)])
                                for mi in range(3):
                                    pending[NSTEP - 3 + mi].append(
                                        (lambda m: lambda:
                                         outproj_partial3(m, at3))(mi))
                                pending[NSTEP + 3].append(
                                    lambda: outproj_partial3(3, at3))
                            elif p_r == NPAIR - 1:
                                at4 = ats.pop((q_h, q_q))
                                if last_q:
                                    for mi, at_idx in ((0, 2), (1, 4),
                                                       (2, 5), (3, 6)):
                                        pending[NSTEP + at_idx].append(
                                            (lambda m: lambda:
                                             outproj_final(m, at4[3]))(mi))
                                else:
                                    for mi in range(4):
                                        for kh in range(2):
                                            pending[base + 5 + 6 * mi +
                                                    3 * kh].append(
                                                (lambda m, h: lambda:
                                                 outproj_part(q_h, q_q,
                                                              at4, m, h))
                                                (mi, kh))
                        return go
                    def mk_a(oc, q_h, q_q, p_r, base):
                        def go_a():
                            otmp, rb = normalize_a(oc)
                            pending[base + 4].append(
                                mk_b(otmp, rb, q_h, q_q, p_r, base))
                        return go_a
                    pending[i + 1].append(mk_a(oacc_done, qh, qq, pr, i))

            def warmer():
                # keep the PE p-state up across the drain's normalize
                # latency (results unused; ps2 slots are free by now)
                dm = ps.tile([DH + 1, 512], F32, tag="ps2", bufs=2,
                             name="warm")
                for j in range(3):
                    nc.tensor.matmul(dm[:], lhsT=v_sb[:, 0, 0, :],
                                     rhs=qt[0][0:P, 0:512],
                                     start=(j == 0), stop=(j == 2),
                                     skip_group_check=True)
            pending[NSTEP].append(warmer)
            pending[NSTEP + 1].append(warmer)

            while pending:
                idx = min(pending)
                for fn in pending.pop(idx):
                    fn()

    nc.compile()
    return nc


def _get_program():
    global _CACHED
    if _CACHED is None:
        _CACHED = _build_program()
    return _CACHED


def make_in_maps(query, key, value, mask, Wq, bq, Wk, bk, Wv, bv, Wo, bo):
    query = np.asarray(query, np.float32)
    key = np.asarray(key, np.float32)
    value = np.asarray(value, np.float32)
    mask = np.asarray(mask)
    Wq = np.asarray(Wq, np.float32)
    Wk = np.asarray(Wk, np.float32)
    Wv = np.asarray(Wv, np.float32)
    Wo = np.asarray(Wo, np.float32)
    in_maps = []
    for c in range(NCORES):
        b, g = c // 2, c % 2
        cols = slice(g * D, (g + 1) * D)
        in_maps.append({
            "xq": np.ascontiguousarray(query[b].T).astype(NPBF16),
            "xk": np.ascontiguousarray(key[b].T).astype(NPBF16),
            "xv": np.ascontiguousarray(value[b].T).astype(NPBF16),
            "maskT": np.ascontiguousarray(mask[b].T).astype(NPBF16),
            "wq": Wq[:, cols].astype(NPBF16),
            "wk": Wk[:, cols].astype(NPBF16),
            "wv": Wv[:, cols].astype(NPBF16),
            "wo": np.ascontiguousarray(Wo[cols, :]).astype(NPBF16),
        })
    return in_maps


def kernel(query, key, value, mask, Wq, bq, Wk, bk, Wv, bv, Wo, bo,
           **unused):
    assert not np.any(np.asarray(bq)) and not np.any(np.asarray(bk)) \
        and not np.any(np.asarray(bv)), "nonzero qkv bias unsupported"
    nc = _get_program()
    in_maps = make_in_maps(query, key, value, mask, Wq, bq, Wk, bk, Wv, bv,
                           Wo, bo)
    res = run_bass_kernel_spmd(nc, in_maps, list(range(NCORES)))
    bo = np.asarray(bo, np.float32)
    outv = np.empty((B, S, HID), np.float32)
    for b in range(B):
        outv[b] = res.results[2 * b]["out"] + res.results[2 * b + 1]["out"] + bo
    return outv


# revision 32
# speedup vs baseline: 1.0040x; 1.0040x over previous
"""Multi-head attention layer (B=4, S=2048, HID=1024, 16 heads) on 8 TRN2 NeuronCores.

Sharding (hardcoded): core c -> (batch b = c//2, head-group g = c%2).
Each core computes its 8 heads' full attention for its batch.

v2 architecture: the kernel is ScalarE-bound (256 exp activations of
[128,1024], ~1.1us each, ~288us total).  Everything else hides under
the exp stream:

  - Attention runs as one flat 256-step pipeline (quarter-major,
    pair-major, key-chunk inner) starting as soon as pair 0's Q/K
    projections land (~25us), instead of after ALL projections (~115us
    in v1).  All remaining work (pairs 1-3 K proj, Q proj split into
    s-quarter units by per-quarter deadline, V proj, output-projection
    strips, mask/weight DMAs) is injected between attention steps as
    4-matmul "filler" parts, ordered by data deadline and sized so a
    part never delays the next E by more than the exp runway.  E for
    step i+3 is emitted before step i's fillers; every consumer is
    emitted after its producer (the Tile framework derives
    dependencies from program order, not runtime order).
  - ScalarE does ONLY exp (plus pair-0 projection evacuations and the
    quarter-0 mask-piece DMA issues in the prologue while it is idle).
    bq/bk/bv are identically zero in this problem's setup_inputs
    (asserted host-side).
  - DVE carries the mask multiplies (2x mode), projection/V/strip
    evacuations, reciprocals, and normalize multiplies -- ~250us,
    under the exp stream.  GPSIMD runs ONLY partition_broadcast:
    mixing op families on Pool triggers ~6-9us microcode library swaps
    (measured), so everything else was moved off it.
  - Normalize per pair is split: part A (oacc -> otmp evacuation,
    denominator row via SBUF->SBUF DMA, reciprocal, partition
    broadcast) at the pair's last step; part B (normalize multiplies +
    rows 64-127 via DMA) three steps later so no engine queue
    head-blocks on the chain, and the next pair's PSUM accumulators
    recycle without stalling exp.
  - Tail: the last quarter's output-projection strips accumulate pairs
    0-2 during the final steps, with only pair 3's contribution +
    evacuation (split across DVE/ACT and both DMA queues) after the
    last exp; dummy ps2 matmuls keep the PE p-state up across the
    drain.  PSUM: 3x [128,1024] rotating (E/proj/V/strips) + 2 banks
    for the PV accumulators ([65,512]; 65th V column of ones yields
    the softmax denominators).

Numerics (exact vs the reference up to float rounding): softmax
without max-subtraction (|scores| <= ~8, exp cannot overflow);
exp * {0,1}-mask == the reference's -1e9 masking; bo added on host;
Wo and normalized probabilities in bf16 (measured 6.7e-3 relative
absmax vs the fp32 reference, gate is 2e-2).
"""

import sys

for _p in ("/opt/trn_rl_repo", "/root/.axon_site/_ro/trn_rl_repo"):
    if _p not in sys.path:
        sys.path.insert(0, _p)

import numpy as np
import ml_dtypes

import concourse.bass as bass
import concourse.tile as tile
from concourse import bacc, mybir
from concourse.bass_utils import run_bass_kernel_spmd

F32 = mybir.dt.float32
BF16 = mybir.dt.bfloat16
NPBF16 = ml_dtypes.bfloat16

B, S, HID = 4, 2048, 1024
HEADS, DH = 16, 64
NCORES = 8
D = 512
HLOC = 8
NPAIR = 4
P = 128
KC = S // P      # 16 key chunks
NKP = HID // P   # 8 contraction chunks
SCALE = 1.0 / 8.0
EXP = mybir.ActivationFunctionType.Exp

PE_BUFS = 5       # pe_t (exp output) elasticity
PM_BUFS = 5       # pm (masked probs) elasticity / PV lag tolerance
AT_BUFS = 6
MASK_BUFS = 4     # [P, 4, 512] quarter-piece mask tiles
POOL_MASK_KCS = ()  # Pool mask-mult offload hurt on HW (PV stalls)
PV_DEFER = 2

_CACHED = None


def _build_program():
    nc = bacc.Bacc("TRN2", target_bir_lowering=False, debug=False,
                   num_devices=NCORES)

    xq = nc.dram_tensor("xq", [HID, S], BF16, kind="ExternalInput").ap()
    xk = nc.dram_tensor("xk", [HID, S], BF16, kind="ExternalInput").ap()
    xv = nc.dram_tensor("xv", [HID, S], BF16, kind="ExternalInput").ap()
    mk = nc.dram_tensor("maskT", [S, S], BF16, kind="ExternalInput").ap()
    wq = nc.dram_tensor("wq", [HID, D], BF16, kind="ExternalInput").ap()
    wk = nc.dram_tensor("wk", [HID, D], BF16, kind="ExternalInput").ap()
    wv = nc.dram_tensor("wv", [HID, D], BF16, kind="ExternalInput").ap()
    wo = nc.dram_tensor("wo", [D, HID], BF16, kind="ExternalInput").ap()
    out = nc.dram_tensor("out", [S, HID], F32, kind="ExternalOutput").ap()

    with tile.TileContext(nc) as tc:
        with tc.tile_pool(name="sb", bufs=1) as sb, \
             tc.tile_pool(name="ps", bufs=1, space="PSUM") as ps:

            # ---------------- persistent SBUF ----------------
            qt = [sb.tile([P, S], BF16, tag="qt", bufs=NPAIR, name=f"qt{p}")
                  for p in range(NPAIR)]
            kt = [sb.tile([P, S], BF16, tag="kt", bufs=NPAIR, name=f"kt{p}")
                  for p in range(NPAIR)]
            v_sb = sb.tile([P, KC, HLOC, DH + 1], BF16, tag="v", name="v_sb")
            nc.vector.memset(v_sb[:, :, :, DH:DH + 1], 1.0)
            wo_sb = sb.tile([P, 4, HID], BF16, tag="wo", name="wo_sb")

            # ---------------- prologue DMAs ----------------
            # Split across BOTH hardware DGE queues (SP + the idle
            # Activation queue) so the lead-in halves.  Each FIFO is
            # ordered by data deadline; pool allocations that could
            # block a FIFO sit at its tail.
            w_t = {}

            def load_w(nm, wd, eng):
                t = sb.tile([P, NKP, D], BF16, tag="w", bufs=3, name=nm)
                eng.dma_start(t[:], wd.rearrange("(c p) d -> p c d", p=P))
                w_t[nm] = t

            x_t = {}

            def load_xhalf(key, xd, sh, eng, split=False):
                t = sb.tile([P, NKP, 1024], BF16, tag="x", bufs=3,
                            name=f"{key}h{sh}")
                view = xd.rearrange("(c p) s -> p c s", p=P)
                if split:
                    # two k-half transfers so the projection's first
                    # parts can start while the second half streams
                    eng.dma_start(t[:, 0:4, :],
                                  view[:, 0:4, sh * 1024:(sh + 1) * 1024])
                    eng.dma_start(t[:, 4:8, :],
                                  view[:, 4:8, sh * 1024:(sh + 1) * 1024])
                else:
                    eng.dma_start(t[:],
                                  view[:, :, sh * 1024:(sh + 1) * 1024])
                x_t[(key, sh)] = t

            masks = {}

            def load_mask(qh, qq, piece, eng):
                mt = sb.tile([P, 4, 512], BF16, tag="mask", bufs=MASK_BUFS,
                             name="mask_sb")
                eng.dma_start(
                    mt[:],
                    mk.rearrange("(kc p) (u q) -> u p kc q", p=P, q=512)
                    [qh * 2 + qq][:, piece * 4:(piece + 1) * 4, :])
                masks[(qh, qq, piece)] = mt

            xv_t = {}

            def load_xv(g, eng):
                t = sb.tile([P, NKP, 512], BF16, tag="xv", bufs=2,
                            name=f"xv{g}")
                eng.dma_start(
                    t[:], xv.rearrange("(c p) s -> p c s", p=P)
                    [:, :, g * 512:(g + 1) * 512])
                xv_t[g] = t

            # SP queue, deadline-ordered.  ACT-queue DMA issues cost
            # ~2.3us of ScalarE each, so only the quarter-0 mask pieces
            # ride that queue (issued before the first exp, when the
            # engine is otherwise idle anyway).
            load_w("wk", wk, nc.sync)
            load_w("wq", wq, nc.sync)
            load_xhalf("xk", xk, 0, nc.sync)
            load_xhalf("xq", xq, 0, nc.sync)
            load_w("wv", wv, nc.sync)
            load_xv(0, nc.sync)
            load_xhalf("xk", xk, 1, nc.sync)
            load_xv(1, nc.sync)
            for piece in range(4):
                load_mask(0, 0, piece, nc.scalar)

            # ---------------- work units ----------------
            proj_state = {}

            def proj_part(wkey, xkey, dst, m, sh, part, evac="dve"):
                """Quarter of a (pair m, s-half sh) projection: 4
                matmuls (n2 = part//2, k-half = part%2); part 3
                evacuates the [128, 1024] tile to dst[m] (bf16)."""
                if part == 0:
                    proj_state[(wkey, m, sh)] = ps.tile(
                        [P, 1024], F32, tag="ps4", bufs=3, name="prps")
                acc = proj_state[(wkey, m, sh)]
                n2, kh = part // 2, part % 2
                for k in range(kh * 4, kh * 4 + 4):
                    nc.tensor.matmul(
                        acc[:, n2 * 512:(n2 + 1) * 512],
                        lhsT=w_t[wkey][:, k, m * P:(m + 1) * P],
                        rhs=x_t[(xkey, sh)][:, k, n2 * 512:(n2 + 1) * 512],
                        start=(k == 0), stop=(k == NKP - 1))
                if part == 3:
                    dstap = dst[m][:, sh * 1024:(sh + 1) * 1024]
                    if evac == "act":
                        nc.scalar.copy(dstap, acc[:])
                    else:
                        nc.vector.tensor_copy(dstap, acc[:])
                    del proj_state[(wkey, m, sh)]

            def proj_sq(wkey, xkey, dst, m, sh, sq, part, evac="dve"):
                """s-quarter Q projection (4 matmuls per part; 2 parts):
                512 output columns, own psum tile, for split deadlines."""
                key = (wkey, m, sh, sq)
                if part == 0:
                    proj_state[key] = ps.tile([P, 1024], F32, tag="ps4",
                                              bufs=3, name="prps")
                acc = proj_state[key]
                for k in range(part * 4, part * 4 + 4):
                    nc.tensor.matmul(
                        acc[:, 0:512],
                        lhsT=w_t[wkey][:, k, m * P:(m + 1) * P],
                        rhs=x_t[(xkey, sh)][:, k,
                                            sq * 512:(sq + 1) * 512],
                        start=(k == 0), stop=(k == NKP - 1))
                if part == 1:
                    c0 = sh * 1024 + sq * 512
                    dstap = dst[m][:, c0:c0 + 512]
                    if evac == "act":
                        nc.scalar.copy(dstap, acc[:, 0:512])
                    else:
                        nc.vector.tensor_copy(dstap, acc[:, 0:512])
                    del proj_state[key]

            v_ps = {}

            def v_chunk_part(m, kh):
                """Half of V-projection s-chunk m (4 matmuls); kh==1
                evacuates the chunk (PV of step kc=m reads it)."""
                slot, half = m // 2, m % 2
                if half == 0 and kh == 0:
                    v_ps[slot] = ps.tile([P, 1024], F32, tag="ps4", bufs=3,
                                         name="vps")
                accv = v_ps[slot]
                g, part = m // 4, m % 4
                for k in range(kh * 4, kh * 4 + 4):
                    nc.tensor.matmul(
                        accv[:, half * 512:(half + 1) * 512],
                        lhsT=xv_t[g][:, k, part * P:(part + 1) * P],
                        rhs=w_t["wv"][:, k, :],
                        start=(k == 0), stop=(k == NKP - 1))
                if kh == 1:
                    nc.vector.tensor_copy(
                        v_sb[:, m, :, 0:DH],
                        accv[:, half * 512:(half + 1) * 512]
                        .rearrange("p (h d) -> p h d", h=HLOC))
                    if half == 1:
                        del v_ps[slot]

            strip_state = {}

            def outproj_part(qh, qq, at4, m, kh):
                q0 = qh * 1024 + qq * 512
                if kh == 0:
                    strip_state[(qh, qq, m)] = ps.tile(
                        [P, HID], F32, tag="ps4", bufs=3, name="ops")
                ops = strip_state[(qh, qq, m)]
                for k in range(kh * 2, kh * 2 + 2):
                    for n2 in range(2):
                        nc.tensor.matmul(
                            ops[:, n2 * 512:(n2 + 1) * 512],
                            lhsT=at4[k][:, m * P:(m + 1) * P],
                            rhs=wo_sb[:, k, n2 * 512:(n2 + 1) * 512],
                            start=(k == 0), stop=(k == 3))
                if kh == 1:
                    ost = sb.tile([P, HID], F32, tag="ost", bufs=2,
                                  name="ost")
                    nc.vector.tensor_copy(ost[:], ops[:])
                    nc.sync.dma_start(out[q0 + m * P: q0 + (m + 1) * P, :],
                                      ost[:])
                    del strip_state[(qh, qq, m)]

            def outproj_partial3(m, at3):
                """Last-quarter strip m: pairs 0-2 accumulation (6
                matmuls), emitted during the final steps once E
                allocations have ceased (ps4 rotation)."""
                ops = ps.tile([P, HID], F32, tag="ps4", bufs=3, name="ops")
                strip_state[("last", m)] = ops
                for k in range(3):
                    for n2 in range(2):
                        nc.tensor.matmul(
                            ops[:, n2 * 512:(n2 + 1) * 512],
                            lhsT=at3[k][:, m * P:(m + 1) * P],
                            rhs=wo_sb[:, k, n2 * 512:(n2 + 1) * 512],
                            start=(k == 0), stop=False,
                            skip_group_check=True)

            def outproj_final(m, at_last):
                q0 = 1024 + 512
                ops = strip_state.pop(("last", m))
                for n2 in range(2):
                    nc.tensor.matmul(
                        ops[:, n2 * 512:(n2 + 1) * 512],
                        lhsT=at_last[:, m * P:(m + 1) * P],
                        rhs=wo_sb[:, 3, n2 * 512:(n2 + 1) * 512],
                        start=False, stop=True, skip_group_check=True)
                ost = sb.tile([P, HID], F32, tag="ost", bufs=2, name="ost")
                nc.vector.tensor_copy(ost[:], ops[:])
                nc.sync.dma_start(out[q0 + m * P: q0 + (m + 1) * P, :],
                                  ost[:])

            # ---------------- normalize (split A/B) ----------------
            def normalize_a(oacc):
                otmp = [sb.tile([DH + 1, 512], F32, tag="otmp", bufs=2,
                                name="otmp") for _ in range(2)]
                for hh in range(2):
                    nc.vector.tensor_copy(otmp[hh][:], oacc[hh][:])
                d0 = sb.tile([1, 1024], F32, tag="d0", bufs=1, name="d0")
                for hh in range(2):
                    nc.sync.dma_start(d0[0:1, hh * 512:(hh + 1) * 512],
                                      otmp[hh][DH:DH + 1, :])
                nc.vector.reciprocal_approx_fast(d0[:], d0[:])
                rb = sb.tile([DH, 1024], F32, tag="rb", bufs=1, name="rb")
                nc.gpsimd.partition_broadcast(rb[:], d0[:], channels=DH)
                return otmp, rb

            def normalize_b(otmp, rb, on_pool=False):
                eng = nc.vector
                at = sb.tile([P, 512], BF16, tag="at", bufs=AT_BUFS,
                             name="at")
                eng.tensor_mul(at[0:DH, :], otmp[0][0:DH, :],
                               rb[:, 0:512])
                tb = sb.tile([DH, 512], BF16, tag="tmpb", bufs=1, name="tb")
                eng.tensor_mul(tb[:], otmp[1][0:DH, :],
                               rb[:, 512:1024])
                nc.sync.dma_start(at[DH:P, :], tb[:])
                return at

            # ---------------- step list & E ----------------
            quarters = [(0, 0), (0, 1), (1, 0), (1, 1)]
            steps = [(qh, qq, pr, kc)
                     for (qh, qq) in quarters
                     for pr in range(NPAIR)
                     for kc in range(KC)]
            NSTEP = len(steps)
            LOOKAHEAD = 3
            eps = {}

            def emit_e(qh, qq, pr, kc):
                q0 = qh * 1024 + qq * 512
                ep = ps.tile([P, 1024], F32, tag="ps4", bufs=3, name="ep")
                for hh in range(2):
                    rows = slice(hh * DH, (hh + 1) * DH)
                    nc.tensor.matmul(
                        ep[:, hh * 512:(hh + 1) * 512],
                        lhsT=kt[pr][rows, kc * P:(kc + 1) * P],
                        rhs=qt[pr][rows, q0:q0 + 512],
                        start=True, stop=True)
                eps[(qh, qq, pr, kc)] = ep

            # ---------------- filler schedule ----------------
            from collections import defaultdict
            fill = defaultdict(list)

            def PU(idx, wkey, xkey, dst, m, sh, evac="dve"):
                """Projection unit as 4 single-step parts at idx..idx+3."""
                for part in range(4):
                    fill[idx + part].append(
                        (lambda p: lambda: proj_part(wkey, xkey, dst, m,
                                                     sh, p, evac))(part))

            def QU(idx, m, sh, sq):
                """Q-proj s-quarter unit: 2 parts at idx, idx+1."""
                for part in range(2):
                    fill[idx + part].append(
                        (lambda p: lambda: proj_sq("wq", "xq", qt, m, sh,
                                                   sq, p))(part))

            # V chunk m: parts at steps m-1, m (PV of step kc=m reads the
            # evac; program order defines the dependency).
            fill[0].append(lambda: v_chunk_part(0, 0))
            fill[0].append(lambda: v_chunk_part(0, 1))
            for m in range(1, KC):
                fill[m - 1].append((lambda mm: lambda: v_chunk_part(mm, 0))(m))
                fill[m].append((lambda mm: lambda: v_chunk_part(mm, 1))(m))
            fill[4].append(lambda: load_xv(2, nc.sync))
            fill[10].append(lambda: load_xv(3, nc.sync))
            # pair-0 s-half-1 K proj (E kc8 of pair 0 is emitted at step 5)
            # keys 512:1024 of pair 0 (E kc4 emitted at step 1)
            fill[0].insert(0, lambda: proj_sq("wk", "xk", kt, 0, 0, 1, 0,
                                              "act"))
            fill[0].insert(1, lambda: proj_sq("wk", "xk", kt, 0, 0, 1, 1,
                                              "act"))
            PU(1, "wk", "xk", kt, 0, 1)
            # pairs 1-3 (E of pair p emitted from step 16p-3; kc8 at 16p+5)
            PU(5, "wk", "xk", kt, 1, 0)        # evac @8 < 13
            QU(11, 1, 0, 0)                    # qt1 q-cols 0:512 by 13
            PU(16, "wk", "xk", kt, 1, 1)       # evac @19 < 21
            PU(20, "wk", "xk", kt, 2, 0)       # evac @23 < 29
            QU(26, 2, 0, 0)                    # by 29
            PU(30, "wk", "xk", kt, 2, 1)       # evac @33 < 37
            PU(36, "wk", "xk", kt, 3, 0)       # evac @39 < 45
            QU(42, 3, 0, 0)                    # by 45
            PU(47, "wk", "xk", kt, 3, 1)       # evac @50 < 53
            # deferred q-cols 512:1024 (quarter (0,1), deadlines 61+16p)
            QU(22, 0, 0, 1)
            QU(33, 1, 0, 1)
            QU(56, 2, 0, 1)
            QU(70, 3, 0, 1)
            # bulk mid-run loads placed in DMA-quiet windows between the
            # per-pair normalize chains
            fill[52].append(lambda: load_xhalf("xq", xq, 1, nc.sync))
            fill[48].append(lambda: nc.sync.dma_start(
                wo_sb[:], wo.rearrange("(c p) n -> p c n", p=P)))
            # deferred Q proj s-half 1 in s-quarters (quarter (1,0)
            # needs q-cols 1024:1536 from step 125+16p; (1,1) cols
            # 1536:2048 from 189+16p)
            for i in range(NPAIR):
                QU(94 + 6 * i, i, 1, 0)
                QU(150 + 6 * i, i, 1, 1)
            # mask quarter-pieces for quarters 1-3 (slot of piece j of
            # the prior quarter frees at step 64(Q-1)+51+4j)
            for Q in range(1, 4):
                qh_, qq_ = quarters[Q]
                for j in range(4):
                    fill[64 * Q - 12 + 4 * j].append(
                        (lambda a, b, c: lambda: load_mask(a, b, c, nc.sync))
                        (qh_, qq_, j))

            # ---------------- prologue PE work ----------------
            for part in range(2):
                proj_sq("wk", "xk", kt, 0, 0, 0, part, "act")
            for part in range(2):
                proj_sq("wq", "xq", qt, 0, 0, 0, part, "act")

            # ---------------- main loop ----------------
            oaccs = {}
            ats = {}
            pending = defaultdict(list)

            for j in range(LOOKAHEAD):
                emit_e(*steps[j])

            def emit_pv(pr, kc, pm_t):
                for hh in range(2):
                    nc.tensor.matmul(
                        oaccs[pr][hh][:],
                        lhsT=v_sb[:, kc, 2 * pr + hh, :],
                        rhs=pm_t[:, hh, :],
                        start=(kc == 0), stop=(kc == KC - 1),
                        skip_group_check=True)

            for i, (qh, qq, pr, kc) in enumerate(steps):
                # E first: filler psum-allocation stalls then only delay
                # E(i+4..), absorbed by the lookahead.  All qt/kt/mask
                # producers are scheduled >= 1 step before the first E
                # emission that reads them.
                if i + LOOKAHEAD < NSTEP:
                    emit_e(*steps[i + LOOKAHEAD])
                for fn in fill.pop(i, ()):
                    fn()
                for fn in pending.pop(i, ()):
                    fn()

                if kc == 0:
                    oaccs[pr] = [ps.tile([DH + 1, 512], F32, tag="ps2",
                                         bufs=2, name="oacc")
                                 for _ in range(2)]

                ep = eps.pop((qh, qq, pr, kc))
                pe_t = sb.tile([P, 1024], BF16, tag="p", bufs=PE_BUFS,
                               name="pexp")
                nc.scalar.activation(pe_t[:], ep[:], EXP, scale=SCALE)
                pm_t = sb.tile([P, 2, 512], BF16, tag="pm", bufs=PM_BUFS,
                               name="pmask")
                mslice = masks[(qh, qq, kc // 4)][:, kc % 4, :]
                eng = (nc.gpsimd if kc in POOL_MASK_KCS else nc.vector)
                eng.tensor_mul(
                    pm_t[:],
                    pe_t[:].rearrange("p (h q) -> p h q", h=2),
                    mslice.unsqueeze(1).to_broadcast([P, 2, 512]))
                if kc in POOL_MASK_KCS:
                    pending[i + PV_DEFER].append(
                        (lambda c, d, t: lambda: emit_pv(c, d, t))
                        (pr, kc, pm_t))
                else:
                    emit_pv(pr, kc, pm_t)

                if kc == KC - 1:
                    otmp, rb = normalize_a(oaccs.pop(pr))

                    def mk_b(o, r, q_h, q_q, p_r, base):
                        last_q = (q_h, q_q) == quarters[-1]

                        def go():
                            at = normalize_b(o, r,
                                             on_pool=not (last_q and
                                                          p_r == 3))
                            ats.setdefault((q_h, q_q), []).append(at)
                            if last_q and p_r == 2:
                                # strips 0-2 partial (pairs 0-2) in the
                                # last 3 steps; strip 3's partial waits
                                # for a ps4 slot freed by final(0).
                                at3 = list(ats[(q_h, q_q)])
                                for mi in range(3):
                                    pending[NSTEP - 3 + mi].append(
                                        (lambda m: lambda:
                                         outproj_partial3(m, at3))(mi))
                                pending[NSTEP + 3].append(
                                    lambda: outproj_partial3(3, at3))
                            elif p_r == NPAIR - 1:
                                at4 = ats.pop((q_h, q_q))
                                if last_q:
                                    for mi, at_idx in ((0, 2), (1, 4),
                                                       (2, 5), (3, 6)):
                                        pending[NSTEP + at_idx].append(
                                            (lambda m: lambda:
                                             outproj_final(m, at4[3]))(mi))
                                else:
                                    for mi in range(4):
                                        for kh in range(2):
                                            pending[base + 4 + 6 * mi +
                                                    3 * kh].append(
                                                (lambda m, h: lambda:
                                                 outproj_part(q_h, q_q,
                                                              at4, m, h))
                                                (mi, kh))
                        return go
                    pending[i + 3].append(mk_b(otmp, rb, qh, qq, pr, i + 1))

            def warmer():
                # keep the PE p-state up across the drain's normalize
                # latency (results unused; ps2 slots are free by now)
                dm = ps.tile([DH + 1, 512], F32, tag="ps2", bufs=2,
                             name="warm")
                for j in range(3):
                    nc.tensor.matmul(dm[:], lhsT=v_sb[:, 0, 0, :],
                                     rhs=qt[0][0:P, 0:512],
                                     start=(j == 0), stop=(j == 2),
                                     skip_group_check=True)
            pending[NSTEP].append(warmer)
            pending[NSTEP + 1].append(warmer)

            while pending:
                idx = min(pending)
                for fn in pending.pop(idx):
                    fn()

    nc.compile()
    return nc


def _get_program():
    global _CACHED
    if _CACHED is None:
        _CACHED = _build_program()
    return _CACHED


def make_in_maps(query, key, value, mask, Wq, bq, Wk, bk, Wv, bv, Wo, bo):
    query = np.asarray(query, np.float32)
    key = np.asarray(key, np.float32)
    value = np.asarray(value, np.float32)
    mask = np.asarray(mask)
    Wq = np.asarray(Wq, np.float32)
    Wk = np.asarray(Wk, np.float32)
    Wv = np.asarray(Wv, np.float32)
    Wo = np.asarray(Wo, np.float32)
    in_maps = []
    for c in range(NCORES):
        b, g = c // 2, c % 2
        cols = slice(g * D, (g + 1) * D)
        in_maps.append({
            "xq": np.ascontiguousarray(query[b].T).astype(NPBF16),
            "xk": np.ascontiguousarray(key[b].T).astype(NPBF16),
            "xv": np.ascontiguousarray(value[b].T).astype(NPBF16),
            "maskT": np.ascontiguousarray(mask[b].T).astype(NPBF16),
            "wq": Wq[:, cols].astype(NPBF16),
            "wk": Wk[:, cols].astype(NPBF16),
            "wv": Wv[:, cols].astype(NPBF16),
            "wo": np.ascontiguousarray(Wo[cols, :]).astype(NPBF16),
        })
    return in_maps


def kernel(query, key, value, mask, Wq, bq, Wk, bk, Wv, bv, Wo, bo,
           **unused):
    assert not np.any(np.asarray(bq)) and not np.any(np.asarray(bk)) \
        and not np.any(np.asarray(bv)), "nonzero qkv bias unsupported"
    nc = _get_program()
    in_maps = make_in_maps(query, key, value, mask, Wq, bq, Wk, bk, Wv, bv,
                           Wo, bo)
    res = run_bass_kernel_spmd(nc, in_maps, list(range(NCORES)))
    bo = np.asarray(bo, np.float32)
    outv = np.empty((B, S, HID), np.float32)
    for b in range(B):
        outv[b] = res.results[2 * b]["out"] + res.results[2 * b + 1]["out"] + bo
    return outv
